# revision 18
# baseline (speedup 1.0000x reference)
"""DFSMN (order-9 IIR + 2-tap lookahead FIR along frames) on 8 Trainium2 cores.

Math: the reference computes, per (b, h, d) sequence along frames t:
    p[t] = base[t] + sum_{k=1..9} c_k[d] * p[t-k]
    base[t] = (1 + l0[d]) v[t] + r1[d] v[t+1] + r2[d] v[t+2]
This is a per-channel LTI filter, so p = w_d * v (convolution with the
filter's impulse response, which decays like rho^n with rho <= ~0.89, below
fp32 resolution past lag ~170). Each 128-frame output block therefore
depends only on the previous 256 input frames, which turns the whole
problem into, per channel d:

    out_block(b) = W1_d^T @ x[window b] + W2_d^T @ x[window b+1]

with W1/W2 128x128 Toeplitz matrices built on the host from the impulse
response, and windows = consecutive 128-frame chunks of the front-padded
input. Because the window offset advances exactly BH free-elements per
block, all 8 blocks collapse into a single FD=512 matmul per term.

Precision/perf: fp32 matmuls on TRN2 run as hi/lo pairs (~5 cyc/row). We
instead split W and x into bf16 hi+lo halves on the host and compute
    W@x ~= Whi@xhi + Whi@xlo + Wlo@xhi        (lo*lo term ~2^-16, dropped)
as 6 bf16 matmuls (1 cyc/row) accumulating in fp32 PSUM: ~2x less PE time,
relative error ~1.5e-5.

The "base does not exist for t<0" boundary (folding the r-taps into W
pretends it does) needs a rank-2 correction on output block 0 that depends
only on v[0:2]; it is precomputed on the host (2x128x64 MACs per channel)
and added on-chip during the PSUM->SBUF copy.

Sharding: channels d (512) split across 8 cores (64 each); all 64 (b,h)
sequences ride the matmul free dimension. Zero cross-device communication.

Per-core inputs (host-prepared):
    x  [64 d, 2 (hi/lo), 1152 t_padded, 64 bh] bf16  (126 zero frames front, 2 back)
    w  [64 d, 128 k, 512] bf16   columns = [W1hi | W2hi | W1lo | W2lo]
    co [64 d, 128, 64] f32       precomputed block-0 boundary correction
    y  [64 d, 1024 t, 64 bh] f32
"""

import numpy as np

import concourse.bass as bass
import concourse.bacc as bacc
import concourse.mybir as mybir
from concourse import tile
from concourse import bass_utils

B, H, T, D = 16, 4, 1024, 512
N_CORES = 8
DC = D // N_CORES          # 64 channels per core
BH = B * H                 # 64 sequences (matmul free dim)
NBLK = T // 128            # 8 output blocks
PADF = 126                 # leading zero frames (window 0 history)
TPAD = PADF + T + 2        # 1152 = 9 * 128 exactly
NWIN = TPAD // 128         # 9 windows
F32 = mybir.dt.float32
BF16 = mybir.dt.bfloat16
FREE = NBLK * BH           # 512, matmul free dim

_NC_CACHE: dict = {}


def _build_nc(dc: int = DC):
    nc = bacc.Bacc("TRN2", target_bir_lowering=False, debug=False)
    x = nc.dram_tensor("x", [dc, 2, TPAD, BH], BF16, kind="ExternalInput")
    w = nc.dram_tensor("w", [dc, 128, 512], BF16, kind="ExternalInput")
    co = nc.dram_tensor("co", [dc, 128, BH], F32, kind="ExternalInput")
    y = nc.dram_tensor("y", [dc, T, BH], F32, kind="ExternalOutput")
    xap, yap = x.ap(), y.ap()

    with tile.TileContext(nc) as tc:
        with tc.tile_pool(name="xp", bufs=6) as xp, \
             tc.tile_pool(name="wp", bufs=6) as wp, \
             tc.tile_pool(name="cp", bufs=6) as cp, \
             tc.tile_pool(name="op", bufs=6) as op, \
             tc.tile_pool(name="pp", bufs=6, space="PSUM") as pp:
            for d in range(dc):
                # [128 t-in-window, (half, win, bh)]
                xt = xp.tile([128, 2 * NWIN * BH], BF16, name="xt")
                src = xap.copy()
                src.ap = src.ap[:0] + [[BH, 128], [TPAD * BH, 2],
                                       [128 * BH, NWIN], [1, BH]]
                src.offset = d * 2 * TPAD * BH
                nc.sync.dma_start(out=xt, in_=src)

                wt = wp.tile([128, 512], BF16, name="wt")
                nc.scalar.dma_start(out=wt, in_=w.ap()[d])
                ct = cp.tile([128, BH], F32, name="ct")
                nc.scalar.dma_start(out=ct, in_=co.ap()[d])

                XH = 0                  # xhi windows start (free index)
                XL = NWIN * BH          # xlo windows start
                ps = pp.tile([128, FREE], F32, name="ps")
                nc.tensor.matmul(ps, lhsT=wt[:, 0:128],
                                 rhs=xt[:, XH:XH + FREE],
                                 start=True, stop=False)          # W1hi@xhi
                nc.tensor.matmul(ps, lhsT=wt[:, 128:256],
                                 rhs=xt[:, XH + BH:XH + BH + FREE],
                                 start=False, stop=False)         # W2hi@xhi
                nc.tensor.matmul(ps, lhsT=wt[:, 0:128],
                                 rhs=xt[:, XL:XL + FREE],
                                 start=False, stop=False)         # W1hi@xlo
                nc.tensor.matmul(ps, lhsT=wt[:, 128:256],
                                 rhs=xt[:, XL + BH:XL + BH + FREE],
                                 start=False, stop=False)         # W2hi@xlo
                nc.tensor.matmul(ps, lhsT=wt[:, 256:384],
                                 rhs=xt[:, XH:XH + FREE],
                                 start=False, stop=False)         # W1lo@xhi
                nc.tensor.matmul(ps, lhsT=wt[:, 384:512],
                                 rhs=xt[:, XH + BH:XH + BH + FREE],
                                 start=False, stop=True)          # W2lo@xhi

                ot = op.tile([128, FREE], F32, name="ot")
                nc.vector.tensor_add(ot[:, 0:BH], ps[:, 0:BH], ct)
                nc.vector.tensor_copy(ot[:, BH:FREE], ps[:, BH:FREE])

                dst = yap.copy()
                dst.ap = dst.ap[:0] + [[BH, 128], [128 * BH, NBLK], [1, BH]]
                dst.offset = d * T * BH
                nc.gpsimd.dma_start(out=dst, in_=ot)
    nc.compile()
    return nc


def _get_nc(dc: int = DC):
    if dc not in _NC_CACHE:
        _NC_CACHE[dc] = _build_nc(dc)
    return _NC_CACHE[dc]


def _hi_lo(a32: np.ndarray):
    """Split fp32 into bf16 hi + bf16 lo with hi + lo ~= a32 (~16 mantissa bits)."""
    import ml_dtypes
    bf = ml_dtypes.bfloat16
    hi = a32.astype(np.float32).astype(bf)
    lo = (a32.astype(np.float32) - hi.astype(np.float32)).astype(bf)
    return hi, lo


def _build_filters(l_filter: np.ndarray, r_filter: np.ndarray):
    """Returns wmat [128, 256, D] float64 (k, i; W1 = [:, :128], W2 = [:, 128:])
    and the rank-2 block-0 boundary correction corr [2, 128, D] float64."""
    c = l_filter[1:].astype(np.float64)            # (9, D) IIR coeffs
    d = c.shape[1]
    a = np.zeros((258, d))
    a[0] = 1.0
    for n in range(1, 258):
        for k in range(1, min(9, n) + 1):
            a[n] += c[k - 1] * a[n - k]
    q0 = 1.0 + l_filter[0].astype(np.float64)
    q1 = r_filter[0].astype(np.float64)
    q2 = r_filter[1].astype(np.float64)

    # wseq[lag + 129] = combined FIR tap at lag, lag in [-129, 253] (0 < -2)
    wseq = np.zeros((383, d))
    for lag in range(-2, 254):
        t = q2 * a[lag + 2]
        if lag + 1 >= 0:
            t = t + q1 * a[lag + 1]
        if lag >= 0:
            t = t + q0 * a[lag]
        wseq[lag + 129] = t

    kk = np.arange(128)[:, None]
    ii = np.arange(128)[None, :]
    w1 = wseq[ii - kk + 255]                       # (128, 128, D)
    w2 = wseq[ii - kk + 127]
    wmat = np.concatenate([w1, w2], axis=1)        # (128, 256, D)

    i1 = np.arange(128)
    corr = np.stack([-(q1[None, :] * a[i1 + 1] + q2[None, :] * a[i1 + 2]),
                     -(q2[None, :] * a[i1 + 1])], axis=0)   # (2, 128, D)
    return wmat, corr


def _make_in_maps(v, l_filter, r_filter, n_cores=N_CORES, dc=DC):
    wmat, corr = _build_filters(l_filter, r_filter)
    vr = np.asarray(v, dtype=np.float32).reshape(BH, T, D)

    whi, wlo = _hi_lo(wmat.astype(np.float32))     # (128, 256, D) each
    # w layout per d: [128 k, 512] = [W1hi | W2hi | W1lo | W2lo]
    wbits = np.concatenate([whi, wlo], axis=1)     # (128, 512, D) bf16

    # correction output, computed host-side in f64: co[i, bh, D]
    co_all = np.einsum("jid,jbd->ibd", corr,
                       vr[:, 0:2, :].transpose(1, 0, 2).astype(np.float64))
    co_all = co_all.astype(np.float32)             # (128, BH, D)

    import ml_dtypes
    bf16 = ml_dtypes.bfloat16
    in_maps = []
    for cid in range(n_cores):
        sl = slice(cid * dc, (cid + 1) * dc)
        xc = vr[:, :, sl].transpose(2, 1, 0)       # (dc, T, BH) f32
        xhi, xlo = _hi_lo(xc)
        xcore = np.zeros((dc, 2, TPAD, BH), bf16)
        xcore[:, 0, PADF:PADF + T, :] = xhi
        xcore[:, 1, PADF:PADF + T, :] = xlo
        in_maps.append({
            "x": xcore,
            "w": np.ascontiguousarray(wbits[:, :, sl].transpose(2, 0, 1)),
            "co": np.ascontiguousarray(co_all[:, :, sl].transpose(2, 0, 1)),
        })
    return in_maps


def kernel(v: np.ndarray, l_filter: np.ndarray, r_filter: np.ndarray,
           **_unused) -> np.ndarray:
    nc = _get_nc(DC)
    in_maps = _make_in_maps(v, l_filter, r_filter)
    res = bass_utils.run_bass_kernel_spmd(nc, in_maps, core_ids=list(range(N_CORES)))
    out = np.empty((D, T, BH), np.float32)
    for cid in range(N_CORES):
        out[cid * DC:(cid + 1) * DC] = res.results[cid]["y"]
    return np.ascontiguousarray(out.transpose(2, 1, 0)).reshape(B, H, T, D)


# revision 20
# speedup vs baseline: 1.1145x; 1.1145x over previous
"""DFSMN (order-9 IIR + 2-tap lookahead FIR along frames) on 8 Trainium2 cores.

Math: the reference computes, per (b, h, d) sequence along frames t:
    p[t] = base[t] + sum_{k=1..9} c_k[d] * p[t-k]
    base[t] = (1 + l0[d]) v[t] + r1[d] v[t+1] + r2[d] v[t+2]
This is a per-channel LTI filter, so p = w_d * v (convolution with the
filter's impulse response, which decays like rho^n with rho <= ~0.89, below
fp32 resolution past lag ~170). Each 128-frame output block therefore
depends only on the previous 256 input frames, which turns the whole
problem into, per channel d:

    out_block(b) = W1_d^T @ x[window b] + W2_d^T @ x[window b+1]

with W1/W2 128x128 Toeplitz matrices built on the host from the impulse
response, and windows = consecutive 128-frame chunks of the front-padded
input. Because the window offset advances exactly BH free-elements per
block, all 8 blocks collapse into a single FD=512 matmul per term.

Precision/perf: fp32 matmuls on TRN2 run as hi/lo pairs (~5 cyc/row). We
instead split W and x into bf16 hi+lo halves on the host and compute
    W@x ~= Whi@xhi + Whi@xlo + Wlo@xhi        (lo*lo term ~2^-16, dropped)
as 6 bf16 matmuls (1 cyc/row) accumulating in fp32 PSUM: ~2x less PE time,
relative error ~1.5e-5.

The "base does not exist for t<0" boundary (folding the r-taps into W
pretends it does) needs a rank-2 correction on output block 0 that depends
only on v[0:2]; it is precomputed on the host (2x128x64 MACs per channel)
and added on-chip during the PSUM->SBUF copy.

Sharding: channels d (512) split across 8 cores (64 each); all 64 (b,h)
sequences ride the matmul free dimension. Zero cross-device communication.

Per-core inputs (host-prepared):
    x  [64 d, 2 (hi/lo), 1152 t_padded, 64 bh] bf16  (126 zero frames front, 2 back)
    w  [64 d, 128 k, 512] bf16   columns = [W1hi | W2hi | W1lo | W2lo]
    co [64 d, 128, 64] f32       precomputed block-0 boundary correction
    y  [64 d, 1024 t, 64 bh] f32
"""

import numpy as np

import concourse.bass as bass
import concourse.bacc as bacc
import concourse.mybir as mybir
from concourse import tile
from concourse import bass_utils

B, H, T, D = 16, 4, 1024, 512
N_CORES = 8
DC = D // N_CORES          # 64 channels per core
BH = B * H                 # 64 sequences (matmul free dim)
NBLK = T // 128            # 8 output blocks
PADF = 126                 # leading zero frames (window 0 history)
TPAD = PADF + T + 2        # 1152 = 9 * 128 exactly
NWIN = TPAD // 128         # 9 windows
F32 = mybir.dt.float32
BF16 = mybir.dt.bfloat16
FREE = NBLK * BH           # 512, matmul free dim

_NC_CACHE: dict = {}


def _build_nc(dc: int = DC):
    nc = bacc.Bacc("TRN2", target_bir_lowering=False, debug=False)
    # hi/lo interleaved innermost so DMA runs stay 256B
    x = nc.dram_tensor("x", [dc, TPAD, BH, 2], BF16, kind="ExternalInput")
    w = nc.dram_tensor("w", [dc, 128, 512], BF16, kind="ExternalInput")
    co = nc.dram_tensor("co", [dc, 128, BH], F32, kind="ExternalInput")
    y = nc.dram_tensor("y", [dc, T, BH], F32, kind="ExternalOutput")
    xap, yap = x.ap(), y.ap()

    with tile.TileContext(nc) as tc:
        with tc.tile_pool(name="xp", bufs=6) as xp, \
             tc.tile_pool(name="wp", bufs=6) as wp, \
             tc.tile_pool(name="cp", bufs=6) as cp, \
             tc.tile_pool(name="op", bufs=6) as op, \
             tc.tile_pool(name="pp", bufs=6, space="PSUM") as pp:
            for d in range(dc):
                # [128 t-in-window, (win, bh, half)]
                xt = xp.tile([128, NWIN * BH * 2], BF16, name="xt")
                src = xap.copy()
                src.ap = src.ap[:0] + [[BH * 2, 128], [128 * BH * 2, NWIN],
                                       [1, BH * 2]]
                src.offset = d * TPAD * BH * 2
                nc.sync.dma_start(out=xt, in_=src)

                wt = wp.tile([128, 512], BF16, name="wt")
                nc.scalar.dma_start(out=wt, in_=w.ap()[d])
                ct = cp.tile([128, BH], F32, name="ct")
                nc.sync.dma_start(out=ct, in_=co.ap()[d])

                xv = xt.rearrange("p (win bh two) -> p win bh two",
                                  win=NWIN, bh=BH, two=2)
                xhi_a = xv[:, 0:NBLK, :, 0]          # hi, windows 0..7
                xhi_b = xv[:, 1:NBLK + 1, :, 0]      # hi, windows 1..8
                xlo_a = xv[:, 0:NBLK, :, 1]
                xlo_b = xv[:, 1:NBLK + 1, :, 1]
                ps = pp.tile([128, FREE], F32, name="ps")
                nc.tensor.matmul(ps, lhsT=wt[:, 0:128], rhs=xhi_a,
                                 start=True, stop=False)          # W1hi@xhi
                nc.tensor.matmul(ps, lhsT=wt[:, 128:256], rhs=xhi_b,
                                 start=False, stop=False)         # W2hi@xhi
                nc.tensor.matmul(ps, lhsT=wt[:, 0:128], rhs=xlo_a,
                                 start=False, stop=False)         # W1hi@xlo
                nc.tensor.matmul(ps, lhsT=wt[:, 128:256], rhs=xlo_b,
                                 start=False, stop=False)         # W2hi@xlo
                nc.tensor.matmul(ps, lhsT=wt[:, 256:384], rhs=xhi_a,
                                 start=False, stop=False)         # W1lo@xhi
                nc.tensor.matmul(ps, lhsT=wt[:, 384:512], rhs=xhi_b,
                                 start=False, stop=True)          # W2lo@xhi

                ot = op.tile([128, FREE], F32, name="ot")
                nc.vector.tensor_add(ot[:, 0:BH], ps[:, 0:BH], ct)
                nc.vector.tensor_copy(ot[:, BH:FREE], ps[:, BH:FREE])

                dst = yap.copy()
                dst.ap = dst.ap[:0] + [[BH, 128], [128 * BH, NBLK], [1, BH]]
                dst.offset = d * T * BH
                nc.gpsimd.dma_start(out=dst, in_=ot)
    nc.compile()
    return nc


def _get_nc(dc: int = DC):
    if dc not in _NC_CACHE:
        _NC_CACHE[dc] = _build_nc(dc)
    return _NC_CACHE[dc]


def _hi_lo(a32: np.ndarray):
    """Split fp32 into bf16 hi + bf16 lo with hi + lo ~= a32 (~16 mantissa bits)."""
    import ml_dtypes
    bf = ml_dtypes.bfloat16
    hi = a32.astype(np.float32).astype(bf)
    lo = (a32.astype(np.float32) - hi.astype(np.float32)).astype(bf)
    return hi, lo


def _build_filters(l_filter: np.ndarray, r_filter: np.ndarray):
    """Returns wmat [128, 256, D] float64 (k, i; W1 = [:, :128], W2 = [:, 128:])
    and the rank-2 block-0 boundary correction corr [2, 128, D] float64."""
    c = l_filter[1:].astype(np.float64)            # (9, D) IIR coeffs
    d = c.shape[1]
    a = np.zeros((258, d))
    a[0] = 1.0
    for n in range(1, 258):
        for k in range(1, min(9, n) + 1):
            a[n] += c[k - 1] * a[n - k]
    q0 = 1.0 + l_filter[0].astype(np.float64)
    q1 = r_filter[0].astype(np.float64)
    q2 = r_filter[1].astype(np.float64)

    # wseq[lag + 129] = combined FIR tap at lag, lag in [-129, 253] (0 < -2)
    wseq = np.zeros((383, d))
    for lag in range(-2, 254):
        t = q2 * a[lag + 2]
        if lag + 1 >= 0:
            t = t + q1 * a[lag + 1]
        if lag >= 0:
            t = t + q0 * a[lag]
        wseq[lag + 129] = t

    kk = np.arange(128)[:, None]
    ii = np.arange(128)[None, :]
    w1 = wseq[ii - kk + 255]                       # (128, 128, D)
    w2 = wseq[ii - kk + 127]
    wmat = np.concatenate([w1, w2], axis=1)        # (128, 256, D)

    i1 = np.arange(128)
    corr = np.stack([-(q1[None, :] * a[i1 + 1] + q2[None, :] * a[i1 + 2]),
                     -(q2[None, :] * a[i1 + 1])], axis=0)   # (2, 128, D)
    return wmat, corr


def _make_in_maps(v, l_filter, r_filter, n_cores=N_CORES, dc=DC):
    wmat, corr = _build_filters(l_filter, r_filter)
    vr = np.asarray(v, dtype=np.float32).reshape(BH, T, D)

    whi, wlo = _hi_lo(wmat.astype(np.float32))     # (128, 256, D) each
    # w layout per d: [128 k, 512] = [W1hi | W2hi | W1lo | W2lo]
    wbits = np.concatenate([whi, wlo], axis=1)     # (128, 512, D) bf16

    # correction output, computed host-side in f64: co[i, bh, D]
    co_all = np.einsum("jid,jbd->ibd", corr,
                       vr[:, 0:2, :].transpose(1, 0, 2).astype(np.float64))
    co_all = co_all.astype(np.float32)             # (128, BH, D)

    import ml_dtypes
    bf16 = ml_dtypes.bfloat16
    in_maps = []
    for cid in range(n_cores):
        sl = slice(cid * dc, (cid + 1) * dc)
        xc = vr[:, :, sl].transpose(2, 1, 0)       # (dc, T, BH) f32
        xhi, xlo = _hi_lo(xc)
        xcore = np.zeros((dc, TPAD, BH, 2), bf16)
        xcore[:, PADF:PADF + T, :, 0] = xhi
        xcore[:, PADF:PADF + T, :, 1] = xlo
        in_maps.append({
            "x": xcore,
            "w": np.ascontiguousarray(wbits[:, :, sl].transpose(2, 0, 1)),
            "co": np.ascontiguousarray(co_all[:, :, sl].transpose(2, 0, 1)),
        })
    return in_maps


def kernel(v: np.ndarray, l_filter: np.ndarray, r_filter: np.ndarray,
           **_unused) -> np.ndarray:
    nc = _get_nc(DC)
    in_maps = _make_in_maps(v, l_filter, r_filter)
    res = bass_utils.run_bass_kernel_spmd(nc, in_maps, core_ids=list(range(N_CORES)))
    out = np.empty((D, T, BH), np.float32)
    for cid in range(N_CORES):
        out[cid * DC:(cid + 1) * DC] = res.results[cid]["y"]
    return np.ascontiguousarray(out.transpose(2, 1, 0)).reshape(B, H, T, D)


# revision 22
# speedup vs baseline: 1.2063x; 1.0824x over previous
"""DFSMN (order-9 IIR + 2-tap lookahead FIR along frames) on 8 Trainium2 cores.

Math: the reference computes, per (b, h, d) sequence along frames t:
    p[t] = base[t] + sum_{k=1..9} c_k[d] * p[t-k]
    base[t] = (1 + l0[d]) v[t] + r1[d] v[t+1] + r2[d] v[t+2]
This is a per-channel LTI filter, so p = w_d * v (convolution with the
filter's impulse response, which decays like rho^n with rho <= ~0.89, below
fp32 resolution past lag ~170). Each 128-frame output block therefore
depends only on the previous 256 input frames, which turns the whole
problem into, per channel d:

    out_block(b) = W1_d^T @ x[window b] + W2_d^T @ x[window b+1]

with W1/W2 128x128 Toeplitz matrices built on the host from the impulse
response, and windows = consecutive 128-frame chunks of the front-padded
input. Because the window offset advances exactly BH free-elements per
block, all 8 blocks collapse into a single FD=512 matmul per term.

Precision/perf: fp32 matmuls on TRN2 run as hi/lo pairs (~5 cyc/row). We
instead split W and x into bf16 hi+lo halves on the host and compute
    W@x ~= Whi@xhi + Whi@xlo + Wlo@xhi        (lo*lo term ~2^-16, dropped)
as 6 bf16 matmuls (1 cyc/row) accumulating in fp32 PSUM: ~2x less PE time,
relative error ~1.5e-5.

The "base does not exist for t<0" boundary (folding the r-taps into W
pretends it does) needs a rank-2 correction on output block 0 that depends
only on v[0:2]; it is precomputed on the host (2x128x64 MACs per channel)
and added on-chip during the PSUM->SBUF copy.

Sharding: channels d (512) split across 8 cores (64 each); all 64 (b,h)
sequences ride the matmul free dimension. Zero cross-device communication.

Per-core inputs (host-prepared):
    x  [64 d, 2 (hi/lo), 1152 t_padded, 64 bh] bf16  (126 zero frames front, 2 back)
    w  [64 d, 128 k, 512] bf16   columns = [W1hi | W2hi | W1lo | W2lo]
    co [64 d, 128, 64] f32       precomputed block-0 boundary correction
    y  [64 d, 1024 t, 64 bh] f32
"""

import numpy as np

import concourse.bass as bass
import concourse.bacc as bacc
import concourse.mybir as mybir
from concourse import tile
from concourse import bass_utils

B, H, T, D = 16, 4, 1024, 512
N_CORES = 8
DC = D // N_CORES          # 64 channels per core
BH = B * H                 # 64 sequences (matmul free dim)
NBLK = T // 128            # 8 output blocks
PADF = 126                 # leading zero frames (window 0 history)
TPAD = PADF + T + 2        # 1152 = 9 * 128 exactly
NWIN = TPAD // 128         # 9 windows
F32 = mybir.dt.float32
BF16 = mybir.dt.bfloat16
FREE = NBLK * BH           # 512, matmul free dim

_NC_CACHE: dict = {}


def _build_nc(dc: int = DC):
    nc = bacc.Bacc("TRN2", target_bir_lowering=False, debug=False)
    # hi/lo interleaved innermost so DMA runs stay 256B
    x = nc.dram_tensor("x", [dc, TPAD, BH, 2], BF16, kind="ExternalInput")
    w = nc.dram_tensor("w", [dc, 128, 512], BF16, kind="ExternalInput")
    co = nc.dram_tensor("co", [dc, 128, BH], F32, kind="ExternalInput")
    y = nc.dram_tensor("y", [dc, T, BH], F32, kind="ExternalOutput")
    xap, yap = x.ap(), y.ap()

    with tile.TileContext(nc) as tc:
        with tc.tile_pool(name="xp", bufs=6) as xp, \
             tc.tile_pool(name="wp", bufs=6) as wp, \
             tc.tile_pool(name="cp", bufs=6) as cp, \
             tc.tile_pool(name="op", bufs=6) as op, \
             tc.tile_pool(name="pp", bufs=6, space="PSUM") as pp:
            wt2 = ct4 = None
            for d in range(dc):
                # [128 t-in-window, (win, bh, half)]
                xt = xp.tile([128, NWIN * BH * 2], BF16, name="xt")
                src = xap.copy()
                src.ap = src.ap[:0] + [[BH * 2, 128], [128 * BH * 2, NWIN],
                                       [1, BH * 2]]
                src.offset = d * TPAD * BH * 2
                nc.sync.dma_start(out=xt, in_=src)

                # weights batched 2 channels / DMA, corrections 4 / DMA
                # (tiny per-channel DMAs have a ~1us fixed cost)
                if d % 2 == 0:
                    wt2 = wp.tile([128, 1024], BF16, name="wt2")
                    wsrc = w.ap().copy()
                    wsrc.ap = wsrc.ap[:0] + [[512, 128], [128 * 512, 2],
                                             [1, 512]]
                    wsrc.offset = d * 128 * 512
                    nc.scalar.dma_start(out=wt2, in_=wsrc)
                if d % 4 == 0:
                    ct4 = cp.tile([128, 4 * BH], F32, name="ct4")
                    csrc = co.ap().copy()
                    csrc.ap = csrc.ap[:0] + [[BH, 128], [128 * BH, 4],
                                             [1, BH]]
                    csrc.offset = d * 128 * BH
                    nc.sync.dma_start(out=ct4, in_=csrc)
                wt = wt2[:, (d % 2) * 512:(d % 2) * 512 + 512]
                ct = ct4[:, (d % 4) * BH:(d % 4) * BH + BH]

                xv = xt.rearrange("p (win bh two) -> p win bh two",
                                  win=NWIN, bh=BH, two=2)
                xhi_a = xv[:, 0:NBLK, :, 0]          # hi, windows 0..7
                xhi_b = xv[:, 1:NBLK + 1, :, 0]      # hi, windows 1..8
                xlo_a = xv[:, 0:NBLK, :, 1]
                xlo_b = xv[:, 1:NBLK + 1, :, 1]
                ps = pp.tile([128, FREE], F32, name="ps")
                nc.tensor.matmul(ps, lhsT=wt[:, 0:128], rhs=xhi_a,
                                 start=True, stop=False)          # W1hi@xhi
                nc.tensor.matmul(ps, lhsT=wt[:, 128:256], rhs=xhi_b,
                                 start=False, stop=False)         # W2hi@xhi
                nc.tensor.matmul(ps, lhsT=wt[:, 0:128], rhs=xlo_a,
                                 start=False, stop=False)         # W1hi@xlo
                nc.tensor.matmul(ps, lhsT=wt[:, 128:256], rhs=xlo_b,
                                 start=False, stop=False)         # W2hi@xlo
                nc.tensor.matmul(ps, lhsT=wt[:, 256:384], rhs=xhi_a,
                                 start=False, stop=False)         # W1lo@xhi
                nc.tensor.matmul(ps, lhsT=wt[:, 384:512], rhs=xhi_b,
                                 start=False, stop=True)          # W2lo@xhi

                ot = op.tile([128, FREE], F32, name="ot")
                nc.vector.tensor_add(ot[:, 0:BH], ps[:, 0:BH], ct)
                if d % 2 == 0:
                    nc.vector.tensor_copy(ot[:, BH:FREE], ps[:, BH:FREE])
                else:
                    nc.scalar.copy(ot[:, BH:FREE], ps[:, BH:FREE])

                dst = yap.copy()
                dst.ap = dst.ap[:0] + [[BH, 128], [128 * BH, NBLK], [1, BH]]
                dst.offset = d * T * BH
                nc.gpsimd.dma_start(out=dst, in_=ot)
    nc.compile()
    return nc


def _get_nc(dc: int = DC):
    if dc not in _NC_CACHE:
        _NC_CACHE[dc] = _build_nc(dc)
    return _NC_CACHE[dc]


def _hi_lo(a32: np.ndarray):
    """Split fp32 into bf16 hi + bf16 lo with hi + lo ~= a32 (~16 mantissa bits)."""
    import ml_dtypes
    bf = ml_dtypes.bfloat16
    hi = a32.astype(np.float32).astype(bf)
    lo = (a32.astype(np.float32) - hi.astype(np.float32)).astype(bf)
    return hi, lo


def _build_filters(l_filter: np.ndarray, r_filter: np.ndarray):
    """Returns wmat [128, 256, D] float64 (k, i; W1 = [:, :128], W2 = [:, 128:])
    and the rank-2 block-0 boundary correction corr [2, 128, D] float64."""
    c = l_filter[1:].astype(np.float64)            # (9, D) IIR coeffs
    d = c.shape[1]
    a = np.zeros((258, d))
    a[0] = 1.0
    for n in range(1, 258):
        for k in range(1, min(9, n) + 1):
            a[n] += c[k - 1] * a[n - k]
    q0 = 1.0 + l_filter[0].astype(np.float64)
    q1 = r_filter[0].astype(np.float64)
    q2 = r_filter[1].astype(np.float64)

    # wseq[lag + 129] = combined FIR tap at lag, lag in [-129, 253] (0 < -2)
    wseq = np.zeros((383, d))
    for lag in range(-2, 254):
        t = q2 * a[lag + 2]
        if lag + 1 >= 0:
            t = t + q1 * a[lag + 1]
        if lag >= 0:
            t = t + q0 * a[lag]
        wseq[lag + 129] = t

    kk = np.arange(128)[:, None]
    ii = np.arange(128)[None, :]
    w1 = wseq[ii - kk + 255]                       # (128, 128, D)
    w2 = wseq[ii - kk + 127]
    wmat = np.concatenate([w1, w2], axis=1)        # (128, 256, D)

    i1 = np.arange(128)
    corr = np.stack([-(q1[None, :] * a[i1 + 1] + q2[None, :] * a[i1 + 2]),
                     -(q2[None, :] * a[i1 + 1])], axis=0)   # (2, 128, D)
    return wmat, corr


def _make_in_maps(v, l_filter, r_filter, n_cores=N_CORES, dc=DC):
    wmat, corr = _build_filters(l_filter, r_filter)
    vr = np.asarray(v, dtype=np.float32).reshape(BH, T, D)

    whi, wlo = _hi_lo(wmat.astype(np.float32))     # (128, 256, D) each
    # w layout per d: [128 k, 512] = [W1hi | W2hi | W1lo | W2lo]
    wbits = np.concatenate([whi, wlo], axis=1)     # (128, 512, D) bf16

    # correction output, computed host-side in f64: co[i, bh, D]
    co_all = np.einsum("jid,jbd->ibd", corr,
                       vr[:, 0:2, :].transpose(1, 0, 2).astype(np.float64))
    co_all = co_all.astype(np.float32)             # (128, BH, D)

    import ml_dtypes
    bf16 = ml_dtypes.bfloat16
    in_maps = []
    for cid in range(n_cores):
        sl = slice(cid * dc, (cid + 1) * dc)
        xc = vr[:, :, sl].transpose(2, 1, 0)       # (dc, T, BH) f32
        xhi, xlo = _hi_lo(xc)
        xcore = np.zeros((dc, TPAD, BH, 2), bf16)
        xcore[:, PADF:PADF + T, :, 0] = xhi
        xcore[:, PADF:PADF + T, :, 1] = xlo
        in_maps.append({
            "x": xcore,
            "w": np.ascontiguousarray(wbits[:, :, sl].transpose(2, 0, 1)),
            "co": np.ascontiguousarray(co_all[:, :, sl].transpose(2, 0, 1)),
        })
    return in_maps


def kernel(v: np.ndarray, l_filter: np.ndarray, r_filter: np.ndarray,
           **_unused) -> np.ndarray:
    nc = _get_nc(DC)
    in_maps = _make_in_maps(v, l_filter, r_filter)
    res = bass_utils.run_bass_kernel_spmd(nc, in_maps, core_ids=list(range(N_CORES)))
    out = np.empty((D, T, BH), np.float32)
    for cid in range(N_CORES):
        out[cid * DC:(cid + 1) * DC] = res.results[cid]["y"]
    return np.ascontiguousarray(out.transpose(2, 1, 0)).reshape(B, H, T, D)


# revision 23
# speedup vs baseline: 1.2271x; 1.0173x over previous
"""DFSMN (order-9 IIR + 2-tap lookahead FIR along frames) on 8 Trainium2 cores.

Math: the reference computes, per (b, h, d) sequence along frames t:
    p[t] = base[t] + sum_{k=1..9} c_k[d] * p[t-k]
    base[t] = (1 + l0[d]) v[t] + r1[d] v[t+1] + r2[d] v[t+2]
This is a per-channel LTI filter, so p = w_d * v (convolution with the
filter's impulse response, which decays like rho^n with rho <= ~0.89, below
fp32 resolution past lag ~170). Each 128-frame output block therefore
depends only on the previous 256 input frames, which turns the whole
problem into, per channel d:

    out_block(b) = W1_d^T @ x[window b] + W2_d^T @ x[window b+1]

with W1/W2 128x128 Toeplitz matrices built on the host from the impulse
response, and windows = consecutive 128-frame chunks of the front-padded
input. Because the window offset advances exactly BH free-elements per
block, all 8 blocks collapse into a single FD=512 matmul per term.

Precision/perf: fp32 matmuls on TRN2 run as hi/lo pairs (~5 cyc/row). We
instead split W and x into bf16 hi+lo halves on the host and compute
    W@x ~= Whi@xhi + Whi@xlo + Wlo@xhi        (lo*lo term ~2^-16, dropped)
as 6 bf16 matmuls (1 cyc/row) accumulating in fp32 PSUM: ~2x less PE time,
relative error ~1.5e-5.

The "base does not exist for t<0" boundary (folding the r-taps into W
pretends it does) needs a rank-2 correction on output block 0 that depends
only on v[0:2]; it is precomputed on the host (2x128x64 MACs per channel)
and added on-chip during the PSUM->SBUF copy.

Sharding: channels d (512) split across 8 cores (64 each); all 64 (b,h)
sequences ride the matmul free dimension. Zero cross-device communication.

Per-core inputs (host-prepared):
    x  [64 d, 2 (hi/lo), 1152 t_padded, 64 bh] bf16  (126 zero frames front, 2 back)
    w  [64 d, 128 k, 512] bf16   columns = [W1hi | W2hi | W1lo | W2lo]
    co [64 d, 128, 64] f32       precomputed block-0 boundary correction
    y  [64 d, 1024 t, 64 bh] f32
"""

import numpy as np

import concourse.bass as bass
import concourse.bacc as bacc
import concourse.mybir as mybir
from concourse import tile
from concourse import bass_utils

B, H, T, D = 16, 4, 1024, 512
N_CORES = 8
DC = D // N_CORES          # 64 channels per core
BH = B * H                 # 64 sequences (matmul free dim)
NBLK = T // 128            # 8 output blocks
PADF = 126                 # leading zero frames (window 0 history)
TPAD = PADF + T + 2        # 1152 = 9 * 128 exactly
NWIN = TPAD // 128         # 9 windows
F32 = mybir.dt.float32
BF16 = mybir.dt.bfloat16
FREE = NBLK * BH           # 512, matmul free dim

_NC_CACHE: dict = {}


def _build_nc(dc: int = DC):
    nc = bacc.Bacc("TRN2", target_bir_lowering=False, debug=False)
    # hi/lo interleaved innermost so DMA runs stay 256B
    x = nc.dram_tensor("x", [dc, TPAD, BH, 2], BF16, kind="ExternalInput")
    w = nc.dram_tensor("w", [dc, 128, 512], BF16, kind="ExternalInput")
    co = nc.dram_tensor("co", [dc, 128, BH], F32, kind="ExternalInput")
    y = nc.dram_tensor("y", [dc, T, BH], F32, kind="ExternalOutput")
    xap, yap = x.ap(), y.ap()

    with tile.TileContext(nc) as tc:
        with tc.tile_pool(name="xp", bufs=6) as xp, \
             tc.tile_pool(name="wp", bufs=6) as wp, \
             tc.tile_pool(name="cp", bufs=6) as cp, \
             tc.tile_pool(name="op", bufs=6) as op, \
             tc.tile_pool(name="pp", bufs=6, space="PSUM") as pp:
            wt2 = ct4 = None
            for d in range(dc):
                # [128 t-in-window, (win, bh, half)]
                xt = xp.tile([128, NWIN * BH * 2], BF16, name="xt")
                src = xap.copy()
                src.ap = src.ap[:0] + [[BH * 2, 128], [128 * BH * 2, NWIN],
                                       [1, BH * 2]]
                src.offset = d * TPAD * BH * 2
                # split bulk loads across both HWDGE rings (SP + ACT)
                (nc.sync if d % 2 == 0 else nc.scalar).dma_start(out=xt, in_=src)

                # weights batched 2 channels / DMA, corrections 4 / DMA
                # (tiny per-channel DMAs have a ~1us fixed cost)
                if d % 2 == 0:
                    wt2 = wp.tile([128, 1024], BF16, name="wt2")
                    wsrc = w.ap().copy()
                    wsrc.ap = wsrc.ap[:0] + [[512, 128], [128 * 512, 2],
                                             [1, 512]]
                    wsrc.offset = d * 128 * 512
                    (nc.scalar if d % 4 == 0 else nc.sync).dma_start(
                        out=wt2, in_=wsrc)
                if d % 4 == 0:
                    ct4 = cp.tile([128, 4 * BH], F32, name="ct4")
                    csrc = co.ap().copy()
                    csrc.ap = csrc.ap[:0] + [[BH, 128], [128 * BH, 4],
                                             [1, BH]]
                    csrc.offset = d * 128 * BH
                    nc.sync.dma_start(out=ct4, in_=csrc)
                wt = wt2[:, (d % 2) * 512:(d % 2) * 512 + 512]
                ct = ct4[:, (d % 4) * BH:(d % 4) * BH + BH]

                xv = xt.rearrange("p (win bh two) -> p win bh two",
                                  win=NWIN, bh=BH, two=2)
                xhi_a = xv[:, 0:NBLK, :, 0]          # hi, windows 0..7
                xhi_b = xv[:, 1:NBLK + 1, :, 0]      # hi, windows 1..8
                xlo_a = xv[:, 0:NBLK, :, 1]
                xlo_b = xv[:, 1:NBLK + 1, :, 1]
                ps = pp.tile([128, FREE], F32, name="ps")
                nc.tensor.matmul(ps, lhsT=wt[:, 0:128], rhs=xhi_a,
                                 start=True, stop=False)          # W1hi@xhi
                nc.tensor.matmul(ps, lhsT=wt[:, 128:256], rhs=xhi_b,
                                 start=False, stop=False)         # W2hi@xhi
                nc.tensor.matmul(ps, lhsT=wt[:, 0:128], rhs=xlo_a,
                                 start=False, stop=False)         # W1hi@xlo
                nc.tensor.matmul(ps, lhsT=wt[:, 128:256], rhs=xlo_b,
                                 start=False, stop=False)         # W2hi@xlo
                nc.tensor.matmul(ps, lhsT=wt[:, 256:384], rhs=xhi_a,
                                 start=False, stop=False)         # W1lo@xhi
                nc.tensor.matmul(ps, lhsT=wt[:, 384:512], rhs=xhi_b,
                                 start=False, stop=True)          # W2lo@xhi

                ot = op.tile([128, FREE], F32, name="ot")
                nc.vector.tensor_add(ot[:, 0:BH], ps[:, 0:BH], ct)
                if d % 2 == 0:
                    nc.vector.tensor_copy(ot[:, BH:FREE], ps[:, BH:FREE])
                else:
                    nc.scalar.copy(ot[:, BH:FREE], ps[:, BH:FREE])

                dst = yap.copy()
                dst.ap = dst.ap[:0] + [[BH, 128], [128 * BH, NBLK], [1, BH]]
                dst.offset = d * T * BH
                nc.gpsimd.dma_start(out=dst, in_=ot)
    nc.compile()
    return nc


def _get_nc(dc: int = DC):
    if dc not in _NC_CACHE:
        _NC_CACHE[dc] = _build_nc(dc)
    return _NC_CACHE[dc]


def _hi_lo(a32: np.ndarray):
    """Split fp32 into bf16 hi + bf16 lo with hi + lo ~= a32 (~16 mantissa bits)."""
    import ml_dtypes
    bf = ml_dtypes.bfloat16
    hi = a32.astype(np.float32).astype(bf)
    lo = (a32.astype(np.float32) - hi.astype(np.float32)).astype(bf)
    return hi, lo


def _build_filters(l_filter: np.ndarray, r_filter: np.ndarray):
    """Returns wmat [128, 256, D] float64 (k, i; W1 = [:, :128], W2 = [:, 128:])
    and the rank-2 block-0 boundary correction corr [2, 128, D] float64."""
    c = l_filter[1:].astype(np.float64)            # (9, D) IIR coeffs
    d = c.shape[1]
    a = np.zeros((258, d))
    a[0] = 1.0
    for n in range(1, 258):
        for k in range(1, min(9, n) + 1):
            a[n] += c[k - 1] * a[n - k]
    q0 = 1.0 + l_filter[0].astype(np.float64)
    q1 = r_filter[0].astype(np.float64)
    q2 = r_filter[1].astype(np.float64)

    # wseq[lag + 129] = combined FIR tap at lag, lag in [-129, 253] (0 < -2)
    wseq = np.zeros((383, d))
    for lag in range(-2, 254):
        t = q2 * a[lag + 2]
        if lag + 1 >= 0:
            t = t + q1 * a[lag + 1]
        if lag >= 0:
            t = t + q0 * a[lag]
        wseq[lag + 129] = t

    kk = np.arange(128)[:, None]
    ii = np.arange(128)[None, :]
    w1 = wseq[ii - kk + 255]                       # (128, 128, D)
    w2 = wseq[ii - kk + 127]
    wmat = np.concatenate([w1, w2], axis=1)        # (128, 256, D)

    i1 = np.arange(128)
    corr = np.stack([-(q1[None, :] * a[i1 + 1] + q2[None, :] * a[i1 + 2]),
                     -(q2[None, :] * a[i1 + 1])], axis=0)   # (2, 128, D)
    return wmat, corr


def _make_in_maps(v, l_filter, r_filter, n_cores=N_CORES, dc=DC):
    wmat, corr = _build_filters(l_filter, r_filter)
    vr = np.asarray(v, dtype=np.float32).reshape(BH, T, D)

    whi, wlo = _hi_lo(wmat.astype(np.float32))     # (128, 256, D) each
    # w layout per d: [128 k, 512] = [W1hi | W2hi | W1lo | W2lo]
    wbits = np.concatenate([whi, wlo], axis=1)     # (128, 512, D) bf16

    # correction output, computed host-side in f64: co[i, bh, D]
    co_all = np.einsum("jid,jbd->ibd", corr,
                       vr[:, 0:2, :].transpose(1, 0, 2).astype(np.float64))
    co_all = co_all.astype(np.float32)             # (128, BH, D)

    import ml_dtypes
    bf16 = ml_dtypes.bfloat16
    in_maps = []
    for cid in range(n_cores):
        sl = slice(cid * dc, (cid + 1) * dc)
        xc = vr[:, :, sl].transpose(2, 1, 0)       # (dc, T, BH) f32
        xhi, xlo = _hi_lo(xc)
        xcore = np.zeros((dc, TPAD, BH, 2), bf16)
        xcore[:, PADF:PADF + T, :, 0] = xhi
        xcore[:, PADF:PADF + T, :, 1] = xlo
        in_maps.append({
            "x": xcore,
            "w": np.ascontiguousarray(wbits[:, :, sl].transpose(2, 0, 1)),
            "co": np.ascontiguousarray(co_all[:, :, sl].transpose(2, 0, 1)),
        })
    return in_maps


def kernel(v: np.ndarray, l_filter: np.ndarray, r_filter: np.ndarray,
           **_unused) -> np.ndarray:
    nc = _get_nc(DC)
    in_maps = _make_in_maps(v, l_filter, r_filter)
    res = bass_utils.run_bass_kernel_spmd(nc, in_maps, core_ids=list(range(N_CORES)))
    out = np.empty((D, T, BH), np.float32)
    for cid in range(N_CORES):
        out[cid * DC:(cid + 1) * DC] = res.results[cid]["y"]
    return np.ascontiguousarray(out.transpose(2, 1, 0)).reshape(B, H, T, D)
